# revision 16
# baseline (speedup 1.0000x reference)
"""Multi-head attention (B=2, S=2048, EMB=1024, H=16) on 8 Trainium2 cores.

Sharding: the 4096 (batch, seq) query tokens are split into 8 chunks of 512;
core c handles batch b = c // 4, query rows [512*(c%4), 512*(c%4+1)).  Each
core computes the K/V projections for its full batch (4x redundant, no
collectives), then attention over all 16 heads for its 512 queries, then the
output projection, writing a disjoint [512, 1024] slice of the output.

Layouts: all activations flow feature-major ("transposed", [emb, token]) and
weights are staged pre-transposed ([e_in, e_out]), pre-cast to bf16, and
pre-tiled to [128, e_in/128, n] on the host so every device DMA is a straight
contiguous copy.  The device does zero transposes:
  - qh/kh projections produce qh_T/kh_T [d, tok]   (lhsT = W.T, rhs = x.T)
  - vh projection produces vh [tok, d]             (lhsT = x.T, rhs = W.T)
  - scores_T [k, q] = kh_T.T @ qh_T                (softmax along partitions)
  - exp via ACT with scale=1/8, no max subtraction (|scores| <~ 7 so exp is
    safely in range; matches reference softmax up to rounding)
  - att_u_T [d+1, q] = vhe.T @ exp_T where vhe has an all-ones 65th column,
    so row 64 accumulates the softmax denominator for free
  - normalize: r = 1/denom (DVE), broadcast across partitions via a
    0-stride DMA, multiply into attT (DVE); deferred one head-pair so the
    reciprocal latency never stalls the in-order PE queue
  - out [q, e_out] = att_T.T @ Wo.T                (token-major, DMA-ready)

Head-dim is 64, so score matmuls are packed two heads per PE pass using
row-tiling (tile_position auto-derived from partition bases 0 / 64).  The
kh projection is interleaved into the attention pair loop: it is pure PE
work with no ACT dependency, which keeps the PE dense (and the HAM clock
warm) while the scalar engine chews through the exp() stream.
"""

import numpy as np
import ml_dtypes

import concourse.bass as bass  # noqa: F401
import concourse.mybir as mybir
import concourse.tile as tile
from concourse import bacc
from concourse.bass_utils import run_bass_kernel_spmd

BF = mybir.dt.bfloat16
F32 = mybir.dt.float32

EMB = 1024
HEADS = 16
HD = EMB // HEADS          # 64
B, S = 2, 2048
N_CORES = 8
QS = (B * S) // N_CORES    # 512 queries per core
P = 128
NE = EMB // P              # 8 emb chunks
NJ = S // P                # 16 key-token chunks
HPAIRS = HEADS // 2        # 8
EXPF = mybir.ActivationFunctionType.Exp
SCALE = 1.0 / np.sqrt(HD)  # 0.125


def _build_nc(with_bv: bool, with_bo: bool):
    from contextlib import ExitStack

    nc = bacc.Bacc(num_devices=N_CORES)
    dp = nc.declare_dram_parameter
    # per-core shards, pre-tiled on host: [128, n_chunks, tokens/features]
    qT = dp("qT", [P, NE, QS], BF, isOutput=False)      # this core's queries
    kTs = dp("kTs", [P, NE, QS], BF, isOutput=False)    # this core's k shard
    vTs = dp("vTs", [P, NE, QS], BF, isOutput=False)    # this core's v shard
    WqT = dp("WqT", [P, NE, EMB], BF, isOutput=False)
    WkT = dp("WkT", [P, NE, EMB], BF, isOutput=False)
    WvT = dp("WvT", [P, NE, EMB], BF, isOutput=False)
    WoT = dp("WoT", [P, NE, EMB], BF, isOutput=False)
    bqp = dp("bqp", [P, NE], F32, isOutput=False)
    bkp = dp("bkp", [P, NE], F32, isOutput=False)
    bvr = dp("bvr", [1, EMB], BF, isOutput=False)
    bor = dp("bor", [1, EMB], BF, isOutput=False)
    out = dp("out", [QS, EMB], F32, isOutput=True)

    NG = N_CORES // B  # 4 cores per batch group
    GROUPS = [list(range(g * NG, (g + 1) * NG)) for g in range(B)]

    # collective bounce buffers (plain DRAM; shared-space needs >4-core groups)
    kh_bounce = nc.dram_tensor("kh_bounce", [P, NE, QS], BF)
    vh_bounce = nc.dram_tensor("vh_bounce", [P, 4, EMB], BF)
    kh_gath = nc.dram_tensor("kh_gath", [NG, P, NE, QS], BF)
    vh_gath = nc.dram_tensor("vh_gath", [NG, P, 4, EMB], BF)

    with tile.TileContext(nc) as tc, ExitStack() as ctx:
        wpool = ctx.enter_context(tc.tile_pool(name="wts", bufs=1))
        apool = ctx.enter_context(tc.tile_pool(name="acts", bufs=1))

        # persistent tiles
        qhT_sb = apool.tile([P, NE, QS], BF, tag="qhT")
        attT_sb = apool.tile([P, HPAIRS, QS], BF, tag="attT")

        bqp_sb = wpool.tile([P, NE], F32, tag="bqp")
        nc.sync.dma_start(bqp_sb[:], bqp[:])
        bkp_sb = wpool.tile([P, NE], F32, tag="bkp")
        nc.sync.dma_start(bkp_sb[:], bkp[:])
        if with_bv or with_bo:
            ones_b = wpool.tile([1, P], BF, tag="onesb")
            nc.vector.memset(ones_b[:], 1.0)
        if with_bv:
            bvr_sb = wpool.tile([1, EMB], BF, tag="bvr")
            nc.sync.dma_start(bvr_sb[:], bvr[:])
        if with_bo:
            bor_sb = wpool.tile([1, EMB], BF, tag="bor")
            nc.sync.dma_start(bor_sb[:], bor[:])

        # ---------------- shard projections + gathers ----------------
        with (
            tc.tile_pool(name="wsh", bufs=1) as wsh,
            tc.tile_pool(name="pps", bufs=4, space="PSUM") as pps,
        ):
            # kh shard first so its gather starts as early as possible
            kTs_sb = wsh.tile([P, NE, QS], BF, tag="kTs")
            nc.sync.dma_start(kTs_sb[:], kTs[:])
            WkT_sb = wsh.tile([P, NE, EMB], BF, tag="WkT")
            nc.sync.dma_start(WkT_sb[:], WkT[:])
            vTs_sb = wsh.tile([P, NE, QS], BF, tag="vTs")
            nc.sync.dma_start(vTs_sb[:], vTs[:])
            WvT_sb = wsh.tile([P, NE, EMB], BF, tag="WvT")
            nc.sync.dma_start(WvT_sb[:], WvT[:])
            qT_sb = wsh.tile([P, NE, QS], BF, tag="qTb")
            nc.sync.dma_start(qT_sb[:], qT[:])
            WqT_sb = wsh.tile([P, NE, EMB], BF, tag="WqT")
            nc.sync.dma_start(WqT_sb[:], WqT[:])

            # kh_T shard [emb_out, 512] -> bounce -> AllGather
            khs_sb = wsh.tile([P, NE, QS], BF, tag="khs")
            for mm in range(NE):
                ps = pps.tile([P, QS], F32, tag="pps")
                for kk in range(NE):
                    nc.tensor.matmul(
                        ps[:],
                        WkT_sb[:, kk, mm * P : (mm + 1) * P],
                        kTs_sb[:, kk, :],
                        start=(kk == 0),
                        stop=(kk == NE - 1),
                    )
                nc.vector.tensor_scalar_add(
                    khs_sb[:, mm, :], ps[:], bkp_sb[:, mm : mm + 1]
                )
            nc.sync.dma_start(kh_bounce[:], khs_sb[:])
            nc.gpsimd.collective_compute(
                "AllGather",
                mybir.AluOpType.bypass,
                replica_groups=GROUPS,
                ins=[kh_bounce[:].opt()],
                outs=[kh_gath[:].opt()],
            )

            # vh shard [tok, emb_out] (token-major) -> bounce -> AllGather
            vhs_sb = wsh.tile([P, 4, EMB], BF, tag="vhs")
            for mi in range(QS // P):
                for nn in range(EMB // QS):
                    ps = pps.tile([P, QS], F32, tag="pps")
                    for kk in range(NE):
                        nc.tensor.matmul(
                            ps[:],
                            vTs_sb[:, kk, mi * P : (mi + 1) * P],
                            WvT_sb[:, kk, nn * QS : (nn + 1) * QS],
                            start=(kk == 0),
                            stop=(kk == NE - 1) and not with_bv,
                        )
                    if with_bv:
                        nc.tensor.matmul(
                            ps[:],
                            ones_b[:, :],
                            bvr_sb[:, nn * QS : (nn + 1) * QS],
                            start=False,
                            stop=True,
                        )
                    nc.vector.tensor_copy(
                        vhs_sb[:, mi, nn * QS : (nn + 1) * QS], ps[:]
                    )
            nc.sync.dma_start(vh_bounce[:], vhs_sb[:])
            nc.gpsimd.collective_compute(
                "AllGather",
                mybir.AluOpType.bypass,
                replica_groups=GROUPS,
                ins=[vh_bounce[:].opt()],
                outs=[vh_gath[:].opt()],
            )

            # qh_T [emb_out, 512] -- overlaps the gathers
            for mm in range(NE):
                ps = pps.tile([P, QS], F32, tag="pps")
                for kk in range(NE):
                    nc.tensor.matmul(
                        ps[:],
                        WqT_sb[:, kk, mm * P : (mm + 1) * P],
                        qT_sb[:, kk, :],
                        start=(kk == 0),
                        stop=(kk == NE - 1),
                    )
                nc.vector.tensor_scalar_add(
                    qhT_sb[:, mm, :], ps[:], bqp_sb[:, mm : mm + 1]
                )

        # ---------------- load gathered kh / vhe ----------------
        khT_blk = []
        vhe_blk = []
        for g in range(NG):
            kb = apool.tile([P, NE, QS], BF, tag=f"khb{g}")
            nc.sync.dma_start(kb[:], kh_gath[g])
            khT_blk.append(kb)
            vb = apool.tile([P, 4, HEADS, HD + 1], BF, tag=f"vhb{g}")
            nc.vector.memset(vb[:, :, :, HD], 1.0)
            nc.sync.dma_start(
                vb[:, :, :, 0:HD],
                vh_gath[g].rearrange("p m (h d) -> p m h d", d=HD),
            )
            vhe_blk.append(vb)

        def kh_l(hp, half, j):
            # lhsT [64, 128] for head pair hp, half 0/1, global k-chunk j
            lo = half * HD
            return khT_blk[j // 4][lo : lo + HD, hp, (j % 4) * P : (j % 4 + 1) * P]

        def vhe_l(h, j):
            return vhe_blk[j // 4][:, j % 4, h, :]

        # ---------------- attention ----------------
        WoT_sb = wpool.tile([P, NE, EMB], BF, tag="WoT")
        nc.sync.dma_start(WoT_sb[:], WoT[:])
        with (
            tc.tile_pool(name="scps", bufs=2, space="PSUM") as scps,
            tc.tile_pool(name="attps", bufs=4, space="PSUM") as attps,
            tc.tile_pool(name="ppool", bufs=20) as ppool,
            tc.tile_pool(name="dpool", bufs=2) as dpool,
        ):

            def normalize_pair(hp, att0, att1):
                """attT = att_u * (1/denom); deferred one pair so the
                reciprocal never stalls the in-order PE queue."""
                for hh, att_ps in ((0, att0), (1, att1)):
                    r_t = dpool.tile([P, QS], F32, tag="rec")
                    r = r_t[HD : HD + 1, :]
                    nc.vector.reciprocal(r, att_ps[HD : HD + 1, :])
                    # hop to lane 0: partition_broadcast replicates lane 0
                    r0 = dpool.tile([1, QS], F32, tag="rec0")
                    nc.sync.dma_start(r0[:], r)
                    rb_sb = dpool.tile([HD, QS], F32, tag="rbsb")
                    nc.gpsimd.partition_broadcast(rb_sb[:], r0[:])
                    if hh == 0:
                        nc.vector.tensor_mul(
                            attT_sb[0:HD, hp, :], att_ps[0:HD, :], rb_sb[:]
                        )
                    else:
                        t1 = dpool.tile([HD, QS], BF, tag="todd")
                        nc.vector.tensor_mul(t1[:], att_ps[0:HD, :], rb_sb[:])
                        nc.sync.dma_start(attT_sb[HD:P, hp, :], t1[:])

            def att_pair(php, patt0, patt1, pprobs):
                for jj in range(NJ // 2):
                    pp0, pp1 = pprobs[jj]
                    for t in range(2):
                        j = 2 * jj + t
                        first = jj == 0 and t == 0
                        last = jj == NJ // 2 - 1 and t == 1
                        nc.tensor.matmul(
                            patt0[:],
                            vhe_l(2 * php, j),
                            pp0[:, t * QS : (t + 1) * QS],
                            start=first,
                            stop=last,
                        )
                        nc.tensor.matmul(
                            patt1[:],
                            vhe_l(2 * php + 1, j),
                            pp1[:, t * QS : (t + 1) * QS],
                            start=first,
                            stop=last,
                        )

            pend_att = None   # (hp, att0, att1, probs) awaiting att matmuls
            pend_norm = None  # (hp, att0, att1) awaiting normalize
            for hp in range(HPAIRS):
                # scores (row-packed, 2 heads) + exp
                probs = []
                for jj in range(NJ // 2):
                    sc0 = scps.tile([P, 2 * QS], F32, tag="sc")
                    sc1 = scps.tile([P, 2 * QS], F32, tag="sc")
                    for t in range(2):
                        j = 2 * jj + t
                        nc.tensor.matmul(
                            sc0[:, t * QS : (t + 1) * QS],
                            kh_l(hp, 0, j),
                            qhT_sb[0:HD, hp, :],
                            start=True,
                            stop=True,
                        )
                        nc.tensor.matmul(
                            sc1[:, t * QS : (t + 1) * QS],
                            kh_l(hp, 1, j),
                            qhT_sb[HD:P, hp, :],
                            start=True,
                            stop=True,
                        )
                    p0 = ppool.tile([P, 2 * QS], BF, tag="probs")
                    nc.scalar.activation(p0[:], sc0[:], EXPF, scale=SCALE)
                    p1 = ppool.tile([P, 2 * QS], BF, tag="probs")
                    nc.scalar.activation(p1[:], sc1[:], EXPF, scale=SCALE)
                    probs.append((p0, p1))

                # att matmuls of the previous pair (its probs are ready)
                if pend_att is not None:
                    att_pair(*pend_att)
                    if pend_norm is not None:
                        normalize_pair(*pend_norm)
                    pend_norm = pend_att[:3]

                att0 = attps.tile([HD + 1, QS], F32, tag="att")
                att1 = attps.tile([HD + 1, QS], F32, tag="att")
                pend_att = (hp, att0, att1, probs)

            # drain the pipeline
            att_pair(*pend_att)
            if pend_norm is not None:
                normalize_pair(*pend_norm)
            normalize_pair(*pend_att[:3])

        # ---------------- output projection ----------------
        with (
            tc.tile_pool(name="ops", bufs=2, space="PSUM") as ops,
            tc.tile_pool(name="osb", bufs=3) as osb,
        ):
            for mq in range(QS // P):
                for nn in range(EMB // QS):
                    ps = ops.tile([P, QS], F32, tag="ops")
                    for hp in range(HPAIRS):
                        nc.tensor.matmul(
                            ps[:],
                            attT_sb[:, hp, mq * P : (mq + 1) * P],
                            WoT_sb[:, hp, nn * QS : (nn + 1) * QS],
                            start=(hp == 0),
                            stop=(hp == HPAIRS - 1) and not with_bo,
                        )
                    if with_bo:
                        nc.tensor.matmul(
                            ps[:],
                            ones_b[:, :],
                            bor_sb[:, nn * QS : (nn + 1) * QS],
                            start=False,
                            stop=True,
                        )
                    ob = osb.tile([P, QS], F32, tag="ob")
                    nc.vector.tensor_copy(ob[:], ps[:])
                    nc.sync.dma_start(
                        out[mq * P : (mq + 1) * P, nn * QS : (nn + 1) * QS], ob[:]
                    )

    nc.finalize()
    return nc


_NC_CACHE: dict = {}


def _get_nc(with_bv: bool, with_bo: bool):
    key = (with_bv, with_bo)
    if key not in _NC_CACHE:
        _NC_CACHE[key] = _build_nc(*key)
    return _NC_CACHE[key]


def _feat_tiled(xT):
    """[EMB, n] -> [128, NE, n] contiguous (feature chunks on partitions)."""
    n = xT.shape[1]
    return np.ascontiguousarray(xT.reshape(NE, P, n).transpose(1, 0, 2))


def _stage(inputs):
    bf = ml_dtypes.bfloat16
    f32 = np.float32

    def arr(name):
        return np.asarray(inputs[name], f32)

    q, k, v = arr("q"), arr("k"), arr("v")
    Wq, Wk, Wv, Wo = arr("Wq"), arr("Wk"), arr("Wv"), arr("Wo")
    bq, bk, bv, bo = arr("bq"), arr("bk"), arr("bv"), arr("bo")

    with_bv = bool(np.any(bv))
    with_bo = bool(np.any(bo))

    def wt(W):  # W.T tiled: [128, NE, EMB] bf16
        return _feat_tiled(np.ascontiguousarray(W.T)).astype(bf)

    def xt(x2d):  # x.T tiled: [128, NE, tokens] bf16
        return _feat_tiled(np.ascontiguousarray(x2d.T)).astype(bf)

    common = {
        "WqT": wt(Wq),
        "WkT": wt(Wk),
        "WvT": wt(Wv),
        "WoT": wt(Wo),
        "bqp": np.ascontiguousarray(bq.reshape(NE, P).T),
        "bkp": np.ascontiguousarray(bk.reshape(NE, P).T),
        "bvr": bv.reshape(1, EMB).astype(bf),
        "bor": bo.reshape(1, EMB).astype(bf),
    }
    in_maps = []
    for c in range(N_CORES):
        b_, g = divmod(c, N_CORES // B)
        m = dict(common)
        sl = slice(g * QS, (g + 1) * QS)
        m["qT"] = xt(q[b_, sl, :])
        m["kTs"] = xt(k[b_, sl, :])
        m["vTs"] = xt(v[b_, sl, :])
        in_maps.append(m)
    return in_maps, with_bv, with_bo


def _assemble(results):
    out = np.empty((B, S, EMB), np.float32)
    for c in range(N_CORES):
        b_, g = divmod(c, N_CORES // B)
        out[b_, g * QS : (g + 1) * QS, :] = results[c]["out"]
    return out


def kernel(**inputs) -> np.ndarray:
    in_maps, with_bv, with_bo = _stage(inputs)
    nc = _get_nc(with_bv, with_bo)
    res = run_bass_kernel_spmd(nc, in_maps, list(range(N_CORES)))
    return _assemble(res.results)


# revision 17
# speedup vs baseline: 1.2990x; 1.2990x over previous
"""Multi-head attention (B=2, S=2048, EMB=1024, H=16) on 8 Trainium2 cores.

Sharding: the 4096 (batch, seq) query tokens are split into 8 chunks of 512;
core c handles batch b = c // 4, query rows [512*(c%4), 512*(c%4+1)).  Each
core computes the K/V projections for its full batch (4x redundant, no
collectives), then attention over all 16 heads for its 512 queries, then the
output projection, writing a disjoint [512, 1024] slice of the output.

Layouts: all activations flow feature-major ("transposed", [emb, token]) and
weights are staged pre-transposed ([e_in, e_out]), pre-cast to bf16, and
pre-tiled to [128, e_in/128, n] on the host so every device DMA is a straight
contiguous copy.  The device does zero transposes:
  - qh/kh projections produce qh_T/kh_T [d, tok]   (lhsT = W.T, rhs = x.T)
  - vh projection produces vh [tok, d]             (lhsT = x.T, rhs = W.T)
  - scores_T [k, q] = kh_T.T @ qh_T                (softmax along partitions)
  - exp via ACT with scale=1/8, no max subtraction (|scores| <~ 7 so exp is
    safely in range; matches reference softmax up to rounding)
  - att_u_T [d+1, q] = vhe.T @ exp_T where vhe has an all-ones 65th column,
    so row 64 accumulates the softmax denominator for free
  - normalize: r = 1/denom (DVE), broadcast across partitions via a
    0-stride DMA, multiply into attT (DVE); deferred one head-pair so the
    reciprocal latency never stalls the in-order PE queue
  - out [q, e_out] = att_T.T @ Wo.T                (token-major, DMA-ready)

Head-dim is 64, so score matmuls are packed two heads per PE pass using
row-tiling (tile_position auto-derived from partition bases 0 / 64).  The
kh projection is interleaved into the attention pair loop: it is pure PE
work with no ACT dependency, which keeps the PE dense (and the HAM clock
warm) while the scalar engine chews through the exp() stream.
"""

import numpy as np
import ml_dtypes

import concourse.bass as bass  # noqa: F401
import concourse.mybir as mybir
import concourse.tile as tile
from concourse import bacc
from concourse.bass_utils import run_bass_kernel_spmd

BF = mybir.dt.bfloat16
F32 = mybir.dt.float32

EMB = 1024
HEADS = 16
HD = EMB // HEADS          # 64
B, S = 2, 2048
N_CORES = 8
QS = (B * S) // N_CORES    # 512 queries per core
P = 128
NE = EMB // P              # 8 emb chunks
NJ = S // P                # 16 key-token chunks
HPAIRS = HEADS // 2        # 8
EXPF = mybir.ActivationFunctionType.Exp
SCALE = 1.0 / np.sqrt(HD)  # 0.125


def _build_nc(with_bv: bool, with_bo: bool):
    from contextlib import ExitStack

    nc = bacc.Bacc(num_devices=N_CORES)
    dp = nc.declare_dram_parameter
    # activations / weights pre-tiled on host: [128, n_chunks, tokens/features]
    qT = dp("qT", [P, NE, QS], BF, isOutput=False)
    kT = dp("kT", [P, NE, S], BF, isOutput=False)
    vT = dp("vT", [P, NE, S], BF, isOutput=False)
    WqT = dp("WqT", [P, NE, EMB], BF, isOutput=False)
    WkT = dp("WkT", [P, NE, EMB], BF, isOutput=False)
    WvT = dp("WvT", [P, NE, EMB], BF, isOutput=False)
    WoT = dp("WoT", [P, NE, EMB], BF, isOutput=False)
    bqp = dp("bqp", [P, NE], F32, isOutput=False)
    bkp = dp("bkp", [P, NE], F32, isOutput=False)
    bvr = dp("bvr", [1, EMB], BF, isOutput=False)
    bor = dp("bor", [1, EMB], BF, isOutput=False)
    out = dp("out", [QS, EMB], F32, isOutput=True)

    with tile.TileContext(nc) as tc, ExitStack() as ctx:
        wpool = ctx.enter_context(tc.tile_pool(name="wts", bufs=1))
        apool = ctx.enter_context(tc.tile_pool(name="acts", bufs=1))

        # persistent tiles
        khT_sb = apool.tile([P, NE, S], BF, tag="khT")
        vhe_sb = apool.tile([P, NJ, HEADS, HD + 1], BF, tag="vhe")
        qhT_sb = apool.tile([P, NE, QS], BF, tag="qhT")
        attT_sb = apool.tile([P, HPAIRS, QS], BF, tag="attT")

        bqp_sb = wpool.tile([P, NE], F32, tag="bqp")
        nc.sync.dma_start(bqp_sb[:], bqp[:])
        bkp_sb = wpool.tile([P, NE], F32, tag="bkp")
        nc.sync.dma_start(bkp_sb[:], bkp[:])
        if with_bv or with_bo:
            ones_b = wpool.tile([1, P], BF, tag="onesb")
            nc.vector.memset(ones_b[:], 1.0)
        if with_bv:
            bvr_sb = wpool.tile([1, EMB], BF, tag="bvr")
            nc.sync.dma_start(bvr_sb[:], bvr[:])
        if with_bo:
            bor_sb = wpool.tile([1, EMB], BF, tag="bor")
            nc.sync.dma_start(bor_sb[:], bor[:])

        # ones column of vhe (denominator accumulator)
        nc.vector.memset(vhe_sb[:, :, :, HD], 1.0)

        # ---------------- qh + vh projections ----------------
        with (
            tc.tile_pool(name="wqv", bufs=1) as wqv,
            tc.tile_pool(name="vin", bufs=2) as vin,
            tc.tile_pool(name="pps", bufs=4, space="PSUM") as pps,
        ):
            # qh first: small chunked loads so the PE starts almost at once
            qT_sb = wqv.tile([P, NE, QS], BF, tag="qTb")
            WqT_sb = wqv.tile([P, NE, EMB], BF, tag="WqT")
            for kk in range(NE):
                nc.sync.dma_start(qT_sb[:, kk, :], qT[:, kk, :])
                nc.sync.dma_start(WqT_sb[:, kk, :], WqT[:, kk, :])
            WvT_sb = wqv.tile([P, NE, EMB], BF, tag="WvT")
            nc.sync.dma_start(WvT_sb[:], WvT[:])

            for mm in range(NE):
                ps = pps.tile([P, QS], F32, tag="pps")
                for kk in range(NE):
                    nc.tensor.matmul(
                        ps[:],
                        WqT_sb[:, kk, mm * P : (mm + 1) * P],
                        qT_sb[:, kk, :],
                        start=(kk == 0),
                        stop=(kk == NE - 1),
                    )
                nc.vector.tensor_scalar_add(
                    qhT_sb[:, mm, :], ps[:], bqp_sb[:, mm : mm + 1]
                )

            # vh [tok, emb_out] -> vhe (ones column preserved)
            for mo in range(S // QS):
                vT_blk = vin.tile([P, NE, QS], BF, tag="vTb")
                nc.sync.dma_start(vT_blk[:], vT[:, :, mo * QS : (mo + 1) * QS])
                for mi in range(QS // P):
                    mm = mo * (QS // P) + mi
                    for nn in range(EMB // QS):
                        ps = pps.tile([P, QS], F32, tag="pps")
                        for kk in range(NE):
                            nc.tensor.matmul(
                                ps[:],
                                vT_blk[:, kk, mi * P : (mi + 1) * P],
                                WvT_sb[:, kk, nn * QS : (nn + 1) * QS],
                                start=(kk == 0),
                                stop=(kk == NE - 1) and not with_bv,
                            )
                        if with_bv:
                            nc.tensor.matmul(
                                ps[:],
                                ones_b[:, :],
                                bvr_sb[:, nn * QS : (nn + 1) * QS],
                                start=False,
                                stop=True,
                            )
                        nc.vector.tensor_copy(
                            vhe_sb[:, mm, nn * 8 : (nn + 1) * 8, 0:HD],
                            ps[:].rearrange("p (h d) -> p h d", d=HD),
                        )

        # kh inputs: only needed from the first attention pair on
        WkT_sb = wpool.tile([P, NE, EMB], BF, tag="WkT")
        nc.sync.dma_start(WkT_sb[:], WkT[:])
        kT_sb = wpool.tile([P, NE, S], BF, tag="kTb")
        nc.sync.dma_start(kT_sb[:], kT[:])
        # Wo: loaded during attention so the output projection never waits
        WoT_sb = wpool.tile([P, NE, EMB], BF, tag="WoT")
        nc.sync.dma_start(WoT_sb[:], WoT[:])

        # ---------------- attention (kh interleaved) ----------------
        with (
            tc.tile_pool(name="scps", bufs=2, space="PSUM") as scps,
            tc.tile_pool(name="attps", bufs=4, space="PSUM") as attps,
            tc.tile_pool(name="ppool", bufs=17) as ppool,
            tc.tile_pool(name="dpool", bufs=5) as dpool,
        ):

            def normalize_pair(hp, att0, att1):
                """attT = att_u * (1/denom); deferred one pair so the
                reciprocal never stalls the in-order PE queue."""
                for hh, att_ps in ((0, att0), (1, att1)):
                    r_t = dpool.tile([P, QS], F32, tag="nrm")
                    r = r_t[HD : HD + 1, :]
                    nc.vector.reciprocal(r, att_ps[HD : HD + 1, :])
                    # hop to lane 0: partition_broadcast replicates lane 0
                    r0 = dpool.tile([1, QS], F32, tag="nrm")
                    nc.sync.dma_start(r0[:], r)
                    rb_sb = dpool.tile([HD, QS], F32, tag="nrm")
                    nc.gpsimd.partition_broadcast(rb_sb[:], r0[:])
                    if hh == 0:
                        nc.vector.tensor_mul(
                            attT_sb[0:HD, hp, :], att_ps[0:HD, :], rb_sb[:]
                        )
                    else:
                        t1 = dpool.tile([HD, QS], BF, tag="nrm")
                        nc.vector.tensor_mul(t1[:], att_ps[0:HD, :], rb_sb[:])
                        nc.sync.dma_start(attT_sb[HD:P, hp, :], t1[:])

            def kh_proj(mm):
                for nn in range(S // QS):
                    ps = scps.tile([P, QS], F32, tag="sc")
                    for kk in range(NE):
                        nc.tensor.matmul(
                            ps[:],
                            WkT_sb[:, kk, mm * P : (mm + 1) * P],
                            kT_sb[:, kk, nn * QS : (nn + 1) * QS],
                            start=(kk == 0),
                            stop=(kk == NE - 1),
                        )
                    nc.vector.tensor_scalar_add(
                        khT_sb[:, mm, nn * QS : (nn + 1) * QS],
                        ps[:],
                        bkp_sb[:, mm : mm + 1],
                    )

            def att_pair(php, patt0, patt1, pprobs):
                for jj in range(NJ // 2):
                    pp0, pp1 = pprobs[jj]
                    for t in range(2):
                        j = 2 * jj + t
                        first = jj == 0 and t == 0
                        last = jj == NJ // 2 - 1 and t == 1
                        nc.tensor.matmul(
                            patt0[:],
                            vhe_sb[:, j, 2 * php, :],
                            pp0[:, t * QS : (t + 1) * QS],
                            start=first,
                            stop=last,
                        )
                        nc.tensor.matmul(
                            patt1[:],
                            vhe_sb[:, j, 2 * php + 1, :],
                            pp1[:, t * QS : (t + 1) * QS],
                            start=first,
                            stop=last,
                        )

            pend_att = None   # (hp, att0, att1, probs) awaiting att matmuls
            pend_norm = None  # (hp, att0, att1) awaiting normalize
            for hp in range(HPAIRS):
                # kh chunk for this pair: pure-PE filler work
                kh_proj(hp)

                # scores (row-packed, 2 heads) + exp
                probs = []
                for jj in range(NJ // 2):
                    sc0 = scps.tile([P, 2 * QS], F32, tag="sc")
                    sc1 = scps.tile([P, 2 * QS], F32, tag="sc")
                    for t in range(2):
                        j = 2 * jj + t
                        nc.tensor.matmul(
                            sc0[:, t * QS : (t + 1) * QS],
                            khT_sb[0:HD, hp, j * P : (j + 1) * P],
                            qhT_sb[0:HD, hp, :],
                            start=True,
                            stop=True,
                        )
                        nc.tensor.matmul(
                            sc1[:, t * QS : (t + 1) * QS],
                            khT_sb[HD:P, hp, j * P : (j + 1) * P],
                            qhT_sb[HD:P, hp, :],
                            start=True,
                            stop=True,
                        )
                    p0 = ppool.tile([P, 2 * QS], BF, tag="probs")
                    nc.scalar.activation(p0[:], sc0[:], EXPF, scale=SCALE)
                    p1 = ppool.tile([P, 2 * QS], BF, tag="probs")
                    nc.scalar.activation(p1[:], sc1[:], EXPF, scale=SCALE)
                    probs.append((p0, p1))

                # att matmuls of the previous pair (its probs are ready)
                if pend_att is not None:
                    att_pair(*pend_att)
                    if pend_norm is not None:
                        normalize_pair(*pend_norm)
                    pend_norm = pend_att[:3]

                att0 = attps.tile([HD + 1, QS], F32, tag="att")
                att1 = attps.tile([HD + 1, QS], F32, tag="att")
                pend_att = (hp, att0, att1, probs)

            # drain the pipeline
            att_pair(*pend_att)
            if pend_norm is not None:
                normalize_pair(*pend_norm)
            normalize_pair(*pend_att[:3])

        # ---------------- output projection ----------------
        with (
            tc.tile_pool(name="ops", bufs=2, space="PSUM") as ops,
            tc.tile_pool(name="osb", bufs=3) as osb,
        ):
            for mq in range(QS // P):
                for nn in range(EMB // QS):
                    ps = ops.tile([P, QS], F32, tag="ops")
                    for hp in range(HPAIRS):
                        nc.tensor.matmul(
                            ps[:],
                            attT_sb[:, hp, mq * P : (mq + 1) * P],
                            WoT_sb[:, hp, nn * QS : (nn + 1) * QS],
                            start=(hp == 0),
                            stop=(hp == HPAIRS - 1) and not with_bo,
                        )
                    if with_bo:
                        nc.tensor.matmul(
                            ps[:],
                            ones_b[:, :],
                            bor_sb[:, nn * QS : (nn + 1) * QS],
                            start=False,
                            stop=True,
                        )
                    ob = osb.tile([P, QS], F32, tag="ob")
                    nc.vector.tensor_copy(ob[:], ps[:])
                    nc.sync.dma_start(
                        out[mq * P : (mq + 1) * P, nn * QS : (nn + 1) * QS], ob[:]
                    )

    nc.finalize()
    return nc


_NC_CACHE: dict = {}


def _get_nc(with_bv: bool, with_bo: bool):
    key = (with_bv, with_bo)
    if key not in _NC_CACHE:
        _NC_CACHE[key] = _build_nc(*key)
    return _NC_CACHE[key]


def _feat_tiled(xT):
    """[EMB, n] -> [128, NE, n] contiguous (feature chunks on partitions)."""
    n = xT.shape[1]
    return np.ascontiguousarray(xT.reshape(NE, P, n).transpose(1, 0, 2))


def _stage(inputs):
    bf = ml_dtypes.bfloat16
    f32 = np.float32

    def arr(name):
        return np.asarray(inputs[name], f32)

    q, k, v = arr("q"), arr("k"), arr("v")
    Wq, Wk, Wv, Wo = arr("Wq"), arr("Wk"), arr("Wv"), arr("Wo")
    bq, bk, bv, bo = arr("bq"), arr("bk"), arr("bv"), arr("bo")

    with_bv = bool(np.any(bv))
    with_bo = bool(np.any(bo))

    def wt(W):  # W.T tiled: [128, NE, EMB] bf16
        return _feat_tiled(np.ascontiguousarray(W.T)).astype(bf)

    def xt(x2d):  # x.T tiled: [128, NE, tokens] bf16
        return _feat_tiled(np.ascontiguousarray(x2d.T)).astype(bf)

    common = {
        "WqT": wt(Wq),
        "WkT": wt(Wk),
        "WvT": wt(Wv),
        "WoT": wt(Wo),
        "bqp": np.ascontiguousarray(bq.reshape(NE, P).T),
        "bkp": np.ascontiguousarray(bk.reshape(NE, P).T),
        "bvr": bv.reshape(1, EMB).astype(bf),
        "bor": bo.reshape(1, EMB).astype(bf),
    }
    kT_b = [xt(k[b_]) for b_ in range(B)]
    vT_b = [xt(v[b_]) for b_ in range(B)]

    in_maps = []
    for c in range(N_CORES):
        b_, g = divmod(c, N_CORES // B)
        m = dict(common)
        m["qT"] = xt(q[b_, g * QS : (g + 1) * QS, :])
        m["kT"] = kT_b[b_]
        m["vT"] = vT_b[b_]
        in_maps.append(m)
    return in_maps, with_bv, with_bo


def _assemble(results):
    out = np.empty((B, S, EMB), np.float32)
    for c in range(N_CORES):
        b_, g = divmod(c, N_CORES // B)
        out[b_, g * QS : (g + 1) * QS, :] = results[c]["out"]
    return out


def kernel(**inputs) -> np.ndarray:
    in_maps, with_bv, with_bo = _stage(inputs)
    nc = _get_nc(with_bv, with_bo)
    res = run_bass_kernel_spmd(nc, in_maps, list(range(N_CORES)))
    return _assemble(res.results)
